# revision 7
# baseline (speedup 1.0000x reference)
"""BERT self-attention kernel for Trainium2, sharded over 8 NeuronCores.

Problem: nn_CustomBertSelfAttention (B=2, S=2048, D=1024, H=16 heads, HD=64).

Sharding: tensor-parallel over heads. Core c owns heads {2c, 2c+1}, i.e.
columns [128c, 128c+128) of Wq/Wk/Wv and of the output. Every core reads the
full hidden_states (transposed + cast to bf16 on the host so the contraction
dim lands on SBUF partitions with dense DMA).

Per-core pipeline (all matmuls bf16 with f32 PSUM accumulation):
  1. Projections: Q^T/K^T/V^T [128, B*S] = W_slice^T @ hidden^T.
  2. V^T is transposed back to V [s, dv] via PE-transpose; each (batch, head)
     unit gets an augmented stationary [V | 1] so the attention matmul
     produces both context and the softmax denominator in one pass. Rows are
     pre-scaled by exp(attention_mask) which folds the additive mask into the
     softmax exactly.
  3. Attention per unit (b, h): scores^T tile [k, q] = K^T_tile^T @ Q^T
     (so no transpose of the probabilities is ever needed), exp on ScalarE
     (scale=1/sqrt(HD) folded in; no max-subtraction — scores are O(5) here
     so exp is safe in f32), then ctx^T[65, q] += [V|1]^T @ P^T accumulated
     over k tiles. Row 64 is the denominator.
  4. Normalize: reciprocal of the denominator row, partition-broadcast,
     multiply, DMA ctx^T [64, S] to DRAM.
Host gathers: out[unit] [64, S] is transposed into the [B, S, D] output.
"""
import sys

sys.path.insert(0, "/opt/trn_rl_repo")

import numpy as np
import ml_dtypes

from concourse import bacc
import concourse.mybir as mybir
from concourse.tile import TileContext
from concourse.masks import make_identity
from concourse.bass_utils import run_bass_kernel_spmd

B, S, D, H, HD = 2, 2048, 1024, 16, 64
N_CORES = 8
HPC = H // N_CORES          # heads per core = 2
DC = D // N_CORES           # output/weight columns per core = 128
BS = B * S                  # 4096
NU = B * HPC                # attention units per core = 4
P = 128
F32 = mybir.dt.float32
BF16 = mybir.dt.bfloat16
KT = S // P                 # 16 k-tiles per unit
ONESW = HD + 1              # V_aug width (V columns + ones column)

_cached_nc = None


def build_nc():
    nc = bacc.Bacc(None, target_bir_lowering=False)

    xT = nc.dram_tensor("xT", [D, BS], BF16, kind="ExternalInput")
    w_in = {
        pr: nc.dram_tensor(f"w{pr}", [D, DC], BF16, kind="ExternalInput")
        for pr in "qkv"
    }
    bqkv = nc.dram_tensor("bqkv", [DC, 3], F32, kind="ExternalInput")
    maskT = nc.dram_tensor("maskT", [S, B], F32, kind="ExternalInput")
    out = nc.dram_tensor("out", [NU, HD, S], F32, kind="ExternalOutput")

    from contextlib import ExitStack

    with TileContext(nc) as tc, ExitStack() as es:
        const = es.enter_context(tc.tile_pool(name="const", bufs=1))
        qkvp = es.enter_context(tc.tile_pool(name="qkv", bufs=1))
        wp = es.enter_context(tc.tile_pool(name="wsb", bufs=1))

        ident = const.tile([P, P], BF16)
        make_identity(nc, ident)
        b_sb = const.tile([DC, 3], F32)
        nc.sync.dma_start(b_sb[:], bqkv[:])
        # mask, transposed so the key dim is on partitions: em[p, 16*b + t]
        mk = const.tile([P, B * KT], F32)
        nc.sync.dma_start(
            mk[:].rearrange("p (b t) -> p b t", b=B),
            maskT[:].rearrange("(t p) b -> p b t", p=P),
        )
        em = const.tile([P, B * KT], F32)
        nc.scalar.activation(em[:], mk[:], mybir.ActivationFunctionType.Exp)

        # Persistent per-core activations
        q_sb = qkvp.tile([P, BS], BF16)       # Q^T: [dq, (b s)]
        k_sb = qkvp.tile([P, BS], BF16)       # K^T
        v_aug = [
            qkvp.tile([P, KT * ONESW], BF16, tag=f"vaug{u}", name=f"vaug{u}")
            for u in range(NU)
        ]

        # Weights: w_sb[pr][:, dt*DC:(dt+1)*DC] is the d-tile dt of W slice
        w_sb = {}
        for pr in "qkv":
            w_sb[pr] = wp.tile([P, (D // P) * DC], BF16, tag=f"w{pr}", name=f"w{pr}sb")
            nc.sync.dma_start(
                w_sb[pr][:].rearrange("p (t n) -> p t n", n=DC),
                w_in[pr][:].rearrange("(t p) n -> p t n", p=P),
            )

        # ---------------- Phase 1: projections ----------------
        SCH = 1024
        with nc.named_scope("proj"):
            with tc.tile_pool(name="xp", bufs=3) as xp, \
                 tc.tile_pool(name="vt", bufs=1) as vtp, \
                 tc.tile_pool(name="projps", bufs=1, space="PSUM") as pp, \
                 tc.tile_pool(name="tps", bufs=2, space="PSUM") as tpp:
                v_t = vtp.tile([P, BS], BF16)  # V^T staging
                for sc in range(BS // SCH):
                    ps = {
                        pr: pp.tile([P, SCH], F32, tag=f"ps{pr}", name=f"ps{pr}")
                        for pr in "qkv"
                    }
                    for dt in range(D // P):
                        xt = xp.tile([P, SCH], BF16)
                        nc.sync.dma_start(
                            xt[:], xT[dt * P:(dt + 1) * P, sc * SCH:(sc + 1) * SCH]
                        )
                        for pr in "qkv":
                            for h2 in range(SCH // 512):
                                nc.tensor.matmul(
                                    ps[pr][:, h2 * 512:(h2 + 1) * 512],
                                    lhsT=w_sb[pr][:, dt * DC:(dt + 1) * DC],
                                    rhs=xt[:, h2 * 512:(h2 + 1) * 512],
                                    start=(dt == 0),
                                    stop=(dt == D // P - 1),
                                )
                    sl = slice(sc * SCH, (sc + 1) * SCH)
                    nc.vector.tensor_scalar_add(q_sb[:, sl], ps["q"][:], b_sb[:, 0:1])
                    nc.vector.tensor_scalar_add(k_sb[:, sl], ps["k"][:], b_sb[:, 1:2])
                    nc.vector.tensor_scalar_add(v_t[:, sl], ps["v"][:], b_sb[:, 2:3])

                # V^T -> V, mask-scaled, into per-unit augmented tiles
                for b in range(B):
                    for kt in range(KT):
                        st = b * KT + kt
                        tp = tpp.tile([P, P], BF16, tag="tp")
                        nc.tensor.transpose(
                            tp[:], v_t[:, st * P:(st + 1) * P], ident[:]
                        )
                        for hl in range(HPC):
                            u = b * HPC + hl
                            nc.vector.tensor_scalar_mul(
                                v_aug[u][:, kt * ONESW:kt * ONESW + HD],
                                tp[:, hl * HD:(hl + 1) * HD],
                                em[:, st:st + 1],
                            )
                for u in range(NU):
                    b = u // HPC
                    # ones columns = exp(mask) directly
                    dst = v_aug[u][:].rearrange("p (t w) -> p t w", w=ONESW)
                    nc.vector.tensor_copy(
                        dst[:, :, HD:HD + 1].squeeze(-1),
                        em[:, b * KT:(b + 1) * KT],
                    )

        # ---------------- Phase 2: attention ----------------
        QH = 1024  # q chunk
        with nc.named_scope("attn"):
            with tc.tile_pool(name="sps", bufs=2, space="PSUM") as sp, \
                 tc.tile_pool(name="cps", bufs=2, space="PSUM") as cp, \
                 tc.tile_pool(name="pt", bufs=3) as ptp, \
                 tc.tile_pool(name="ob", bufs=2) as obp, \
                 tc.tile_pool(name="nrm", bufs=2) as nrmp:
                for u in range(NU):
                    b, hl = u // HPC, u % HPC
                    hp = slice(hl * HD, (hl + 1) * HD)
                    bs0 = b * S
                    for qh in range(S // QH):
                        q0 = bs0 + qh * QH
                        cps = cp.tile([ONESW, QH], F32, tag="cps")
                        for kt in range(KT):
                            sps = sp.tile([P, QH], F32, tag="sps")
                            for h2 in range(QH // 512):
                                nc.tensor.matmul(
                                    sps[:, h2 * 512:(h2 + 1) * 512],
                                    lhsT=k_sb[hp, bs0 + kt * P:bs0 + (kt + 1) * P],
                                    rhs=q_sb[hp, q0 + h2 * 512:q0 + (h2 + 1) * 512],
                                    start=True,
                                    stop=True,
                                )
                            pt = ptp.tile([P, QH], BF16, tag="pt")
                            nc.scalar.activation(
                                pt[:], sps[:],
                                mybir.ActivationFunctionType.Exp,
                                scale=float(1.0 / np.sqrt(HD)),
                            )
                            for h2 in range(QH // 512):
                                nc.tensor.matmul(
                                    cps[:, h2 * 512:(h2 + 1) * 512],
                                    lhsT=v_aug[u][:, kt * ONESW:(kt + 1) * ONESW],
                                    rhs=pt[:, h2 * 512:(h2 + 1) * 512],
                                    start=(kt == 0),
                                    stop=(kt == KT - 1),
                                )
                        # normalize rows 0..63 by row 64, write out
                        rc = nrmp.tile([1, QH], F32, tag="rc")
                        nc.vector.reciprocal(rc[:], cps[HD:HD + 1, :])
                        bc = nrmp.tile([HD, QH], F32, tag="bc")
                        nc.gpsimd.partition_broadcast(bc[:], rc[:], channels=HD)
                        o = obp.tile([HD, QH], F32, tag="o")
                        nc.vector.tensor_mul(o[:], cps[0:HD, :], bc[:])
                        nc.sync.dma_start(
                            out[u, :, qh * QH:(qh + 1) * QH], o[:]
                        )

    nc.compile()
    return nc


def _prep_in_maps(hidden_states, attention_mask, Wq, bq, Wk, bk, Wv, bv):
    bf = ml_dtypes.bfloat16
    hs = np.asarray(hidden_states, dtype=np.float32).reshape(BS, D)
    xT = np.ascontiguousarray(hs.T).astype(bf)
    maskT = np.ascontiguousarray(
        np.asarray(attention_mask, dtype=np.float32).reshape(B, S).T
    )
    Ws = {"q": np.asarray(Wq, np.float32), "k": np.asarray(Wk, np.float32),
          "v": np.asarray(Wv, np.float32)}
    bs = {"q": np.asarray(bq, np.float32), "k": np.asarray(bk, np.float32),
          "v": np.asarray(bv, np.float32)}
    in_maps = []
    for c in range(N_CORES):
        sl = slice(c * DC, (c + 1) * DC)
        m = {"xT": xT, "maskT": maskT}
        for pr in "qkv":
            m[f"w{pr}"] = np.ascontiguousarray(Ws[pr][:, sl]).astype(bf)
        m["bqkv"] = np.ascontiguousarray(
            np.stack([bs["q"][sl], bs["k"][sl], bs["v"][sl]], axis=1)
        )
        in_maps.append(m)
    return in_maps


def _gather(results):
    full = np.empty((B, S, D), dtype=np.float32)
    for c in range(N_CORES):
        o = results[c]["out"]  # [NU, HD, S]
        for b in range(B):
            for hl in range(HPC):
                col = c * DC + hl * HD
                full[b, :, col:col + HD] = o[b * HPC + hl].T
    return full


def kernel(hidden_states, attention_mask, Wq, bq, Wk, bk, Wv, bv, **run_kwargs):
    global _cached_nc
    if _cached_nc is None:
        _cached_nc = build_nc()
    in_maps = _prep_in_maps(
        hidden_states, attention_mask, Wq, bq, Wk, bk, Wv, bv
    )
    res = run_bass_kernel_spmd(
        _cached_nc, in_maps, core_ids=list(range(N_CORES)), **run_kwargs
    )
    full = _gather(res.results)
    if run_kwargs:
        kernel.last_result = res
    return full


# revision 8
# speedup vs baseline: 1.0204x; 1.0204x over previous
"""BERT self-attention kernel for Trainium2, sharded over 8 NeuronCores.

Problem: nn_CustomBertSelfAttention (B=2, S=2048, D=1024, H=16 heads, HD=64).

Sharding: tensor-parallel over heads. Core c owns heads {2c, 2c+1}, i.e.
columns [128c, 128c+128) of Wq/Wk/Wv and of the output. Every core reads the
full hidden_states (transposed + cast to bf16 on the host so the contraction
dim lands on SBUF partitions with dense DMA).

Per-core pipeline (all matmuls bf16 with f32 PSUM accumulation):
  1. Projections: Q^T/K^T/V^T [128, chunk] = W_slice^T @ hidden^T, emitted
     per batch so attention for batch 0 starts while batch 1 still projects
     (keeps TensorE dense -> HAM stays at K=8/8).
  2. V^T is PE-transposed back to V [s, dv]; each (batch, head) unit gets an
     augmented stationary [V | exp(mask)] so the attention matmul produces
     both context and the softmax denominator in one pass (the mask scaling
     folds the additive attention mask into softmax exactly).
  3. Attention per unit (b, h): scores^T tile [k, q] = K^T_tile^T @ Q^T
     (no transpose of probabilities needed), exp on ScalarE with the
     1/sqrt(HD) scale folded in (no max-subtraction; scores are O(5) here),
     ctx^T[65, q] += [V|1]^T @ P^T accumulated over k tiles; row 64 is the
     denominator.
  4. Normalize: reciprocal of denominator row, gpsimd partition-broadcast,
     multiply, DMA ctx^T [64, S] out.
Host gathers: out[unit] [64, S] transposed into the [B, S, D] output.

PSUM budget (8 banks): "work" pool 3 x [128,1024] f32 (6 banks) shared by
projection accumulators, V-transpose tiles and score tiles; "cps" pool
1 x [65,1024] (2 banks) for the ctx accumulator.
"""
import sys

sys.path.insert(0, "/opt/trn_rl_repo")

import numpy as np
import ml_dtypes

from concourse import bacc
import concourse.mybir as mybir
from concourse.tile import TileContext
from concourse.masks import make_identity
from concourse.bass_utils import run_bass_kernel_spmd

B, S, D, H, HD = 2, 2048, 1024, 16, 64
N_CORES = 8
HPC = H // N_CORES          # heads per core = 2
DC = D // N_CORES           # output/weight columns per core = 128
BS = B * S                  # 4096
NU = B * HPC                # attention units per core = 4
P = 128
F32 = mybir.dt.float32
BF16 = mybir.dt.bfloat16
KT = S // P                 # 16 k-tiles per unit
ONESW = HD + 1              # V_aug width (V columns + ones column)
SCH = 1024                  # projection s-chunk
QH = 1024                   # attention q-chunk

_cached_nc = None


def build_nc():
    nc = bacc.Bacc(None, target_bir_lowering=False)

    xT = nc.dram_tensor("xT", [D, BS], BF16, kind="ExternalInput")
    w_in = {
        pr: nc.dram_tensor(f"w{pr}", [D, DC], BF16, kind="ExternalInput")
        for pr in "qkv"
    }
    bqkv = nc.dram_tensor("bqkv", [DC, 3], F32, kind="ExternalInput")
    maskT = nc.dram_tensor("maskT", [S, B], F32, kind="ExternalInput")
    out = nc.dram_tensor("out", [NU, HD, S], F32, kind="ExternalOutput")

    from contextlib import ExitStack

    with TileContext(nc) as tc, ExitStack() as es:
        const = es.enter_context(tc.tile_pool(name="const", bufs=1))
        qkvp = es.enter_context(tc.tile_pool(name="qkv", bufs=1))
        wp = es.enter_context(tc.tile_pool(name="wsb", bufs=1))
        xp = es.enter_context(tc.tile_pool(name="xp", bufs=4))
        work = es.enter_context(tc.tile_pool(name="work", bufs=3, space="PSUM"))
        cp = es.enter_context(tc.tile_pool(name="cp", bufs=1, space="PSUM"))
        ptp = es.enter_context(tc.tile_pool(name="ptp", bufs=3))
        obp = es.enter_context(tc.tile_pool(name="obp", bufs=2))
        nrmp = es.enter_context(tc.tile_pool(name="nrmp", bufs=2))

        ident = const.tile([P, P], BF16)
        make_identity(nc, ident)
        b_sb = const.tile([DC, 3], F32)
        nc.sync.dma_start(b_sb[:], bqkv[:])
        # mask with key dim on partitions: em[p, KT*b + t] = exp(mask[b, t*128+p])
        mk = const.tile([P, B * KT], F32)
        nc.sync.dma_start(
            mk[:].rearrange("p (b t) -> p b t", b=B),
            maskT[:].rearrange("(t p) b -> p b t", p=P),
        )
        em = const.tile([P, B * KT], F32)
        nc.scalar.activation(em[:], mk[:], mybir.ActivationFunctionType.Exp)

        # Persistent per-core activations (split per batch for dep granularity)
        q_sb = [qkvp.tile([P, S], BF16, name=f"qsb{b}") for b in range(B)]
        k_sb = [qkvp.tile([P, S], BF16, name=f"ksb{b}") for b in range(B)]
        v_t = [qkvp.tile([P, SCH], BF16, name=f"vt{i}") for i in range(BS // SCH)]
        v_aug = [
            qkvp.tile([P, KT * ONESW], BF16, name=f"vaug{u}") for u in range(NU)
        ]

        # Weights: w_sb[pr][:, dt*DC:(dt+1)*DC] is d-tile dt of the W slice
        w_sb = {}
        for pr in "qkv":
            w_sb[pr] = wp.tile([P, (D // P) * DC], BF16, name=f"w{pr}sb")
            nc.sync.dma_start(
                w_sb[pr][:].rearrange("p (t n) -> p t n", n=DC),
                w_in[pr][:].rearrange("(t p) n -> p t n", p=P),
            )

        def proj_chunk(sc):
            """Project s-chunk sc (1024 positions) into Q^T/K^T/V^T."""
            b, half = (sc * SCH) // S, (sc * SCH) % S
            ps = {}
            for pr in "qkv":
                ps[pr] = work.tile([P, SCH], F32, tag="work", name=f"ps{pr}{sc}")
            for dt in range(D // P):
                xt = xp.tile([P, SCH], BF16, tag="xt", name=f"xt{sc}_{dt}")
                nc.sync.dma_start(
                    xt[:], xT[dt * P:(dt + 1) * P, sc * SCH:(sc + 1) * SCH]
                )
                for pr in "qkv":
                    for h2 in range(SCH // 512):
                        nc.tensor.matmul(
                            ps[pr][:, h2 * 512:(h2 + 1) * 512],
                            lhsT=w_sb[pr][:, dt * DC:(dt + 1) * DC],
                            rhs=xt[:, h2 * 512:(h2 + 1) * 512],
                            start=(dt == 0),
                            stop=(dt == D // P - 1),
                        )
            sl = slice(half, half + SCH)
            nc.vector.tensor_scalar_add(q_sb[b][:, sl], ps["q"][:], b_sb[:, 0:1])
            nc.vector.tensor_scalar_add(k_sb[b][:, sl], ps["k"][:], b_sb[:, 1:2])
            nc.vector.tensor_scalar_add(v_t[sc][:], ps["v"][:], b_sb[:, 2:3])

        def build_vaug(b):
            """PE-transpose V^T of batch b into per-unit [V | exp(mask)]."""
            for kt in range(KT):
                st = b * KT + kt
                tp = work.tile([P, P], BF16, tag="work", name=f"tp{st}")
                vsrc = v_t[st // (SCH // P)]
                off = (st % (SCH // P)) * P
                nc.tensor.transpose(tp[:], vsrc[:, off:off + P], ident[:])
                for hl in range(HPC):
                    u = b * HPC + hl
                    nc.vector.tensor_scalar_mul(
                        v_aug[u][:, kt * ONESW:kt * ONESW + HD],
                        tp[:, hl * HD:(hl + 1) * HD],
                        em[:, st:st + 1],
                    )
            for hl in range(HPC):
                u = b * HPC + hl
                dst = v_aug[u][:].rearrange("p (t w) -> p t w", w=ONESW)
                nc.vector.tensor_copy(
                    dst[:, :, HD:HD + 1].squeeze(-1),
                    em[:, b * KT:(b + 1) * KT],
                )

        def attn_unit(u):
            b, hl = u // HPC, u % HPC
            hp = slice(hl * HD, (hl + 1) * HD)
            for qh in range(S // QH):
                q0 = qh * QH
                cps = cp.tile([ONESW, QH], F32, tag="cps", name=f"cps{u}_{qh}")
                for kt in range(KT):
                    sps = work.tile([P, QH], F32, tag="work", name=f"sps{u}{qh}{kt}")
                    for h2 in range(QH // 512):
                        nc.tensor.matmul(
                            sps[:, h2 * 512:(h2 + 1) * 512],
                            lhsT=k_sb[b][hp, kt * P:(kt + 1) * P],
                            rhs=q_sb[b][hp, q0 + h2 * 512:q0 + (h2 + 1) * 512],
                            start=True,
                            stop=True,
                        )
                    pt = ptp.tile([P, QH], BF16, tag="pt", name=f"pt{u}{qh}{kt}")
                    nc.scalar.activation(
                        pt[:], sps[:],
                        mybir.ActivationFunctionType.Exp,
                        scale=float(1.0 / np.sqrt(HD)),
                    )
                    for h2 in range(QH // 512):
                        nc.tensor.matmul(
                            cps[:, h2 * 512:(h2 + 1) * 512],
                            lhsT=v_aug[u][:, kt * ONESW:(kt + 1) * ONESW],
                            rhs=pt[:, h2 * 512:(h2 + 1) * 512],
                            start=(kt == 0),
                            stop=(kt == KT - 1),
                        )
                rc = nrmp.tile([1, QH], F32, tag="rc", name=f"rc{u}{qh}")
                nc.vector.reciprocal(rc[:], cps[HD:HD + 1, :])
                bc = nrmp.tile([HD, QH], F32, tag="bc", name=f"bc{u}{qh}")
                nc.gpsimd.partition_broadcast(bc[:], rc[:], channels=HD)
                o = obp.tile([HD, QH], F32, tag="o", name=f"o{u}{qh}")
                nc.vector.tensor_mul(o[:], cps[0:HD, :], bc[:])
                nc.sync.dma_start(out[u, :, q0:q0 + QH], o[:])

        # Emission order = scheduler priority: batch-0 projections first, then
        # its attention; batch-1 projections slot under batch-0's ACT-bound
        # attention so TensorE never idles long enough to re-throttle.
        proj_chunk(0)
        proj_chunk(1)
        build_vaug(0)
        attn_unit(0)
        proj_chunk(2)
        attn_unit(1)
        proj_chunk(3)
        build_vaug(1)
        attn_unit(2)
        attn_unit(3)

    nc.compile()
    return nc


def _prep_in_maps(hidden_states, attention_mask, Wq, bq, Wk, bk, Wv, bv):
    bf = ml_dtypes.bfloat16
    hs = np.asarray(hidden_states, dtype=np.float32).reshape(BS, D)
    xT = np.ascontiguousarray(hs.T).astype(bf)
    maskT = np.ascontiguousarray(
        np.asarray(attention_mask, dtype=np.float32).reshape(B, S).T
    )
    Ws = {"q": np.asarray(Wq, np.float32), "k": np.asarray(Wk, np.float32),
          "v": np.asarray(Wv, np.float32)}
    bs = {"q": np.asarray(bq, np.float32), "k": np.asarray(bk, np.float32),
          "v": np.asarray(bv, np.float32)}
    in_maps = []
    for c in range(N_CORES):
        sl = slice(c * DC, (c + 1) * DC)
        m = {"xT": xT, "maskT": maskT}
        for pr in "qkv":
            m[f"w{pr}"] = np.ascontiguousarray(Ws[pr][:, sl]).astype(bf)
        m["bqkv"] = np.ascontiguousarray(
            np.stack([bs["q"][sl], bs["k"][sl], bs["v"][sl]], axis=1)
        )
        in_maps.append(m)
    return in_maps


def _gather(results):
    full = np.empty((B, S, D), dtype=np.float32)
    for c in range(N_CORES):
        o = results[c]["out"]  # [NU, HD, S]
        for b in range(B):
            for hl in range(HPC):
                col = c * DC + hl * HD
                full[b, :, col:col + HD] = o[b * HPC + hl].T
    return full


def kernel(hidden_states, attention_mask, Wq, bq, Wk, bk, Wv, bv, **run_kwargs):
    global _cached_nc
    if _cached_nc is None:
        _cached_nc = build_nc()
    in_maps = _prep_in_maps(
        hidden_states, attention_mask, Wq, bq, Wk, bk, Wv, bv
    )
    res = run_bass_kernel_spmd(
        _cached_nc, in_maps, core_ids=list(range(N_CORES)), **run_kwargs
    )
    full = _gather(res.results)
    if run_kwargs:
        kernel.last_result = res
    return full


# revision 9
# speedup vs baseline: 1.0308x; 1.0102x over previous
"""BERT self-attention kernel for Trainium2, sharded over 8 NeuronCores.

Problem: nn_CustomBertSelfAttention (B=2, S=2048, D=1024, H=16 heads, HD=64).

Sharding: tensor-parallel over heads. Core c owns heads {2c, 2c+1}, i.e.
columns [128c, 128c+128) of Wq/Wk/Wv and of the output. Every core reads the
full hidden_states (transposed + cast to bf16 on the host so the contraction
dim lands on SBUF partitions with dense DMA).

Per-core pipeline (all matmuls bf16 with f32 PSUM accumulation):
  1. Projections: Q^T/K^T/V^T [128, chunk] = W_slice^T @ hidden^T, emitted
     per batch so attention for batch 0 starts while batch 1 still projects
     (keeps TensorE dense -> HAM stays at K=8/8).
  2. V^T is PE-transposed back to V [s, dv]; each (batch, head) unit gets an
     augmented stationary [V | exp(mask)] so the attention matmul produces
     both context and the softmax denominator in one pass (the mask scaling
     folds the additive attention mask into softmax exactly).
  3. Attention per unit (b, h): scores^T tile [k, q] = K^T_tile^T @ Q^T
     (no transpose of probabilities needed), exp on ScalarE with the
     1/sqrt(HD) scale folded in (no max-subtraction; scores are O(5) here),
     ctx^T[65, q] += [V|1]^T @ P^T accumulated over k tiles; row 64 is the
     denominator.
  4. Normalize: reciprocal of denominator row, gpsimd partition-broadcast,
     multiply, DMA ctx^T [64, S] out.
Host gathers: out[unit] [64, S] transposed into the [B, S, D] output.

PSUM budget (8 banks): "work" pool 3 x [128,1024] f32 (6 banks) shared by
projection accumulators, V-transpose tiles and score tiles; "cps" pool
1 x [65,1024] (2 banks) for the ctx accumulator.
"""
import sys

sys.path.insert(0, "/opt/trn_rl_repo")

import numpy as np
import ml_dtypes

from concourse import bacc
import concourse.mybir as mybir
from concourse.tile import TileContext
from concourse.masks import make_identity
from concourse.bass_utils import run_bass_kernel_spmd

B, S, D, H, HD = 2, 2048, 1024, 16, 64
N_CORES = 8
HPC = H // N_CORES          # heads per core = 2
DC = D // N_CORES           # output/weight columns per core = 128
BS = B * S                  # 4096
NU = B * HPC                # attention units per core = 4
P = 128
F32 = mybir.dt.float32
BF16 = mybir.dt.bfloat16
KT = S // P                 # 16 k-tiles per unit
ONESW = HD + 1              # V_aug width (V columns + ones column)
SCH = 1024                  # projection s-chunk
QH = 1024                   # attention q-chunk

_cached_nc = None


def build_nc():
    nc = bacc.Bacc(None, target_bir_lowering=False)

    xT = nc.dram_tensor("xT", [D, BS], BF16, kind="ExternalInput")
    w_in = {
        pr: nc.dram_tensor(f"w{pr}", [D, DC], BF16, kind="ExternalInput")
        for pr in "qkv"
    }
    bqkv = nc.dram_tensor("bqkv", [DC, 3], F32, kind="ExternalInput")
    maskT = nc.dram_tensor("maskT", [S, B], F32, kind="ExternalInput")
    out = nc.dram_tensor("out", [NU, HD, S], F32, kind="ExternalOutput")

    from contextlib import ExitStack

    with TileContext(nc) as tc, ExitStack() as es:
        const = es.enter_context(tc.tile_pool(name="const", bufs=1))
        qkvp = es.enter_context(tc.tile_pool(name="qkv", bufs=1))
        wp = es.enter_context(tc.tile_pool(name="wsb", bufs=1))
        xp = es.enter_context(tc.tile_pool(name="xp", bufs=4))
        work = es.enter_context(tc.tile_pool(name="work", bufs=3, space="PSUM"))
        cp = es.enter_context(tc.tile_pool(name="cp", bufs=1, space="PSUM"))
        ptp = es.enter_context(tc.tile_pool(name="ptp", bufs=3))
        obp = es.enter_context(tc.tile_pool(name="obp", bufs=2))
        nrmp = es.enter_context(tc.tile_pool(name="nrmp", bufs=2))

        ident = const.tile([P, P], BF16)
        make_identity(nc, ident)
        b_sb = const.tile([DC, 3], F32)
        nc.sync.dma_start(b_sb[:], bqkv[:])
        # mask with key dim on partitions: em[p, KT*b + t] = exp(mask[b, t*128+p])
        mk = const.tile([P, B * KT], F32)
        nc.sync.dma_start(
            mk[:].rearrange("p (b t) -> p b t", b=B),
            maskT[:].rearrange("(t p) b -> p b t", p=P),
        )
        em = const.tile([P, B * KT], F32)
        nc.scalar.activation(em[:], mk[:], mybir.ActivationFunctionType.Exp)

        # Persistent per-core activations (split per batch for dep granularity)
        q_sb = [qkvp.tile([P, S], BF16, name=f"qsb{b}") for b in range(B)]
        k_sb = [qkvp.tile([P, S], BF16, name=f"ksb{b}") for b in range(B)]
        v_t = [qkvp.tile([P, SCH], BF16, name=f"vt{i}") for i in range(BS // SCH)]
        v_aug = [
            qkvp.tile([P, KT * ONESW], BF16, name=f"vaug{u}") for u in range(NU)
        ]

        # Weights: w_sb[pr][:, dt*DC:(dt+1)*DC] is d-tile dt of the W slice
        w_sb = {}
        for pr in "qkv":
            w_sb[pr] = wp.tile([P, (D // P) * DC], BF16, name=f"w{pr}sb")
            nc.sync.dma_start(
                w_sb[pr][:].rearrange("p (t n) -> p t n", n=DC),
                w_in[pr][:].rearrange("(t p) n -> p t n", p=P),
            )

        def proj_chunk(sc):
            """Project s-chunk sc (1024 positions) into Q^T/K^T/V^T."""
            b, half = (sc * SCH) // S, (sc * SCH) % S
            ps = {}
            for pr in "qkv":
                ps[pr] = work.tile([P, SCH], F32, tag="work", name=f"ps{pr}{sc}")
            for dt in range(D // P):
                xt = xp.tile([P, SCH], BF16, tag="xt", name=f"xt{sc}_{dt}")
                nc.sync.dma_start(
                    xt[:], xT[dt * P:(dt + 1) * P, sc * SCH:(sc + 1) * SCH]
                )
                for pr in "qkv":
                    for h2 in range(SCH // 512):
                        nc.tensor.matmul(
                            ps[pr][:, h2 * 512:(h2 + 1) * 512],
                            lhsT=w_sb[pr][:, dt * DC:(dt + 1) * DC],
                            rhs=xt[:, h2 * 512:(h2 + 1) * 512],
                            start=(dt == 0),
                            stop=(dt == D // P - 1),
                        )
            sl = slice(half, half + SCH)
            nc.vector.tensor_scalar_add(q_sb[b][:, sl], ps["q"][:], b_sb[:, 0:1])
            nc.vector.tensor_scalar_add(k_sb[b][:, sl], ps["k"][:], b_sb[:, 1:2])
            nc.vector.tensor_scalar_add(v_t[sc][:], ps["v"][:], b_sb[:, 2:3])

        def build_vaug(b):
            """PE-transpose V^T of batch b into per-unit [V | exp(mask)]."""
            for kt in range(KT):
                st = b * KT + kt
                tp = work.tile([P, P], BF16, tag="work", name=f"tp{st}")
                vsrc = v_t[st // (SCH // P)]
                off = (st % (SCH // P)) * P
                nc.tensor.transpose(tp[:], vsrc[:, off:off + P], ident[:])
                for hl in range(HPC):
                    u = b * HPC + hl
                    nc.vector.tensor_scalar_mul(
                        v_aug[u][:, kt * ONESW:kt * ONESW + HD],
                        tp[:, hl * HD:(hl + 1) * HD],
                        em[:, st:st + 1],
                    )
            for hl in range(HPC):
                u = b * HPC + hl
                dst = v_aug[u][:].rearrange("p (t w) -> p t w", w=ONESW)
                nc.vector.tensor_copy(
                    dst[:, :, HD:HD + 1].squeeze(-1),
                    em[:, b * KT:(b + 1) * KT],
                )

        def attn_unit(u):
            b, hl = u // HPC, u % HPC
            hp = slice(hl * HD, (hl + 1) * HD)
            for qh in range(S // QH):
                q0 = qh * QH
                cps = cp.tile([ONESW, QH], F32, tag="cps", name=f"cps{u}_{qh}")
                for kt in range(KT):
                    sps = work.tile([P, QH], F32, tag="work", name=f"sps{u}{qh}{kt}")
                    for h2 in range(QH // 512):
                        nc.tensor.matmul(
                            sps[:, h2 * 512:(h2 + 1) * 512],
                            lhsT=k_sb[b][hp, kt * P:(kt + 1) * P],
                            rhs=q_sb[b][hp, q0 + h2 * 512:q0 + (h2 + 1) * 512],
                            start=True,
                            stop=True,
                        )
                    pt = ptp.tile([P, QH], BF16, tag="pt", name=f"pt{u}{qh}{kt}")
                    nc.scalar.activation(
                        pt[:], sps[:],
                        mybir.ActivationFunctionType.Exp,
                        scale=float(1.0 / np.sqrt(HD)),
                    )
                    for h2 in range(QH // 512):
                        nc.tensor.matmul(
                            cps[:, h2 * 512:(h2 + 1) * 512],
                            lhsT=v_aug[u][:, kt * ONESW:(kt + 1) * ONESW],
                            rhs=pt[:, h2 * 512:(h2 + 1) * 512],
                            start=(kt == 0),
                            stop=(kt == KT - 1),
                        )
                # evacuate PSUM fast (frees the bank for the next qh), then
                # normalize off the PE critical path
                cc = obp.tile([ONESW, QH], F32, tag="cc", name=f"cc{u}{qh}")
                nc.vector.tensor_copy(cc[:], cps[:])
                rc = nrmp.tile([1, QH], F32, tag="rc", name=f"rc{u}{qh}")
                nc.vector.reciprocal(rc[:], cc[HD:HD + 1, :])
                bc = nrmp.tile([HD, QH], F32, tag="bc", name=f"bc{u}{qh}")
                nc.gpsimd.partition_broadcast(bc[:], rc[:], channels=HD)
                o = obp.tile([HD, QH], F32, tag="o", name=f"o{u}{qh}")
                nc.vector.tensor_mul(o[:], cc[0:HD, :], bc[:])
                nc.sync.dma_start(out[u, :, q0:q0 + QH], o[:])

        # Emission order = scheduler priority: batch-0 projections first, then
        # its attention; batch-1 projections slot under batch-0's ACT-bound
        # attention so TensorE never idles long enough to re-throttle.
        proj_chunk(0)
        proj_chunk(1)
        build_vaug(0)
        attn_unit(0)
        proj_chunk(2)
        attn_unit(1)
        proj_chunk(3)
        build_vaug(1)
        attn_unit(2)
        attn_unit(3)

    nc.compile()
    return nc


def _prep_in_maps(hidden_states, attention_mask, Wq, bq, Wk, bk, Wv, bv):
    bf = ml_dtypes.bfloat16
    hs = np.asarray(hidden_states, dtype=np.float32).reshape(BS, D)
    xT = np.ascontiguousarray(hs.T).astype(bf)
    maskT = np.ascontiguousarray(
        np.asarray(attention_mask, dtype=np.float32).reshape(B, S).T
    )
    Ws = {"q": np.asarray(Wq, np.float32), "k": np.asarray(Wk, np.float32),
          "v": np.asarray(Wv, np.float32)}
    bs = {"q": np.asarray(bq, np.float32), "k": np.asarray(bk, np.float32),
          "v": np.asarray(bv, np.float32)}
    in_maps = []
    for c in range(N_CORES):
        sl = slice(c * DC, (c + 1) * DC)
        m = {"xT": xT, "maskT": maskT}
        for pr in "qkv":
            m[f"w{pr}"] = np.ascontiguousarray(Ws[pr][:, sl]).astype(bf)
        m["bqkv"] = np.ascontiguousarray(
            np.stack([bs["q"][sl], bs["k"][sl], bs["v"][sl]], axis=1)
        )
        in_maps.append(m)
    return in_maps


def _gather(results):
    full = np.empty((B, S, D), dtype=np.float32)
    for c in range(N_CORES):
        o = results[c]["out"]  # [NU, HD, S]
        for b in range(B):
            for hl in range(HPC):
                col = c * DC + hl * HD
                full[b, :, col:col + HD] = o[b * HPC + hl].T
    return full


def kernel(hidden_states, attention_mask, Wq, bq, Wk, bk, Wv, bv, **run_kwargs):
    global _cached_nc
    if _cached_nc is None:
        _cached_nc = build_nc()
    in_maps = _prep_in_maps(
        hidden_states, attention_mask, Wq, bq, Wk, bk, Wv, bv
    )
    res = run_bass_kernel_spmd(
        _cached_nc, in_maps, core_ids=list(range(N_CORES)), **run_kwargs
    )
    full = _gather(res.results)
    if run_kwargs:
        kernel.last_result = res
    return full
